# revision 1
# baseline (speedup 1.0000x reference)
"""Batched COO SpMM (gnn_message_passing) for 8 TRN2 NeuronCores.

out[k, i, :] = sum_{e: row[e]==i} values[k, e] * b[k, col[e], :]
  indices [2, 800000] int32, values [4, 800000] f32, b [4, 50000, 64] f32.

Design (pure data parallel over output rows, batch-fused element layout):
  - b_t[node, k*64+f] = b[k, node, f]: 1KB per node row, so ONE dma_gather
    descriptor fetches an edge's source features for all 4 batches.
  - 8 cores x 6250-row octiles. Tokens (edges) are grouped by 128-row output
    window; per window, tokens split into col-bank sections (int16 gather
    index limit), col-sorted, padded to multiples of 128.
  - Per chunk: dma_gather from b_t -> gt [128 tokens/col-group, C, 256];
    DVE multiplies each token block by its per-batch edge values; per
    128-token column DVE builds a one-hot lhsT[t, m] = (relrow[t] == m) and
    PE accumulates psum[m,:] += lhsT^T @ gt[:,c,:] over the window's columns.
  - Window end: PSUM -> SBUF -> DMA to out_t rows. No scatter, no RMW.
  - Pad tokens: gather node 0, values 0, relrow -1 (one-hot row all zero).

The chunk structure is data-dependent (degree distribution) but identical
across cores (SPMD): section sizes are maxima across cores.
"""
import hashlib

import numpy as np

N_NODES = 50000
NNZ = 800000
BATCH = 4
FEAT = 64
ELEM = BATCH * FEAT
N_CORES = 8
ROWS_PER_CORE = N_NODES // N_CORES  # 6250
BANK = 32768
W = 128  # output rows per PSUM window

_cache = {}


# ---------------------------------------------------------------- host prep
def _make_structure(per_core_edges):
    n_cores = len(per_core_edges)
    NW = -(-ROWS_PER_CORE // W)
    core_sections = []
    for rows_local, cols in per_core_edges:
        sections = []
        win = rows_local // W
        order = np.argsort(win, kind="stable")
        bounds = np.searchsorted(win[order], np.arange(NW + 1))
        for w in range(NW):
            in_w = order[bounds[w] : bounds[w + 1]]
            cw = cols[in_w]
            a = in_w[cw < BANK]
            b = in_w[cw >= BANK]
            a = a[np.argsort(cols[a], kind="stable")]
            b = b[np.argsort(cols[b], kind="stable")]
            sections.append((a, b))
        core_sections.append(sections)

    chunks = []
    for w in range(NW):
        nA = max(len(core_sections[c][w][0]) for c in range(n_cores))
        nB = max(len(core_sections[c][w][1]) for c in range(n_cores))
        nA = max(-(-nA // 128) * 128, 128)
        nB = -(-nB // 128) * 128
        parts = [(w, 0, nA)] + ([(w, 1, nB)] if nB else [])
        for i, (w_, b_, n_) in enumerate(parts):
            chunks.append((w_, b_, n_, i == 0, i == len(parts) - 1))

    per_core_tokens = []
    for c, (rows_local, cols) in enumerate(per_core_edges):
        g_parts, r_parts, e_parts = [], [], []
        for w, bank_b, n, _, _ in chunks:
            sel = core_sections[c][w][bank_b]
            k = len(sel)
            g = np.zeros(n, np.int16)
            rr = np.full(n, -1.0, np.float32)
            e = np.full(n, -1, np.int64)
            g[:k] = (cols[sel] - (BANK if bank_b else 0)).astype(np.int16)
            rr[:k] = (rows_local[sel] - w * W).astype(np.float32)
            e[:k] = sel
            g_parts.append(g)
            r_parts.append(rr)
            e_parts.append(e)
        per_core_tokens.append(
            {
                "g": np.concatenate(g_parts),
                "rr": np.concatenate(r_parts),
                "e": np.concatenate(e_parts),
            }
        )
    return chunks, per_core_tokens


def _pack_core_inputs(tokens, values_be, chunks):
    g_cols, r_cols, v_cols = [], [], []
    off = 0
    for _, _, n, _, _ in chunks:
        g = tokens["g"][off : off + n]
        rr = tokens["rr"][off : off + n]
        e = tokens["e"][off : off + n]
        off += n
        g_cols.append(g.reshape(-1, 16).T)
        r_cols.append(rr.reshape(-1, 128).T)
        v = np.zeros((n, BATCH), np.float32)
        real = e >= 0
        v[real] = values_be[:, e[real]].T
        v_cols.append(v.reshape(-1, 128, BATCH).transpose(1, 0, 2))
    g_idx = np.tile(np.concatenate(g_cols, axis=1), (8, 1)).astype(np.int16)
    relrow = np.concatenate(r_cols, axis=1).astype(np.float32)
    vals = np.ascontiguousarray(np.concatenate(v_cols, axis=1).astype(np.float32))
    return {
        "g_idx": np.ascontiguousarray(g_idx),
        "relrow": np.ascontiguousarray(relrow),
        "vals": vals,
    }


# HW constraint: a single dma_gather instruction with more than ~1300
# descriptors crashes the core (NRT_EXEC_UNIT_UNRECOVERABLE). Split large
# gathers into sub-instructions of at most GCAP tokens.
GCAP = 1024


# ---------------------------------------------------------------- device code
def _build(chunks):
    import concourse.bacc as bacc
    import concourse.bass as bass
    import concourse.mybir as mybir
    import concourse.tile as tile

    f32 = mybir.dt.float32
    i16 = mybir.dt.int16
    T = sum(c[2] for c in chunks)
    S_total, C_total = T // 16, T // 128
    R = ROWS_PER_CORE

    nc = bacc.Bacc(None, target_bir_lowering=False)
    b_t = nc.dram_tensor("b_t", [N_NODES, ELEM], f32, kind="ExternalInput")
    g_idx = nc.dram_tensor("g_idx", [128, S_total], i16, kind="ExternalInput")
    relrow = nc.dram_tensor("relrow", [128, C_total], f32, kind="ExternalInput")
    vals = nc.dram_tensor("vals", [128, C_total, BATCH], f32, kind="ExternalInput")
    out_t = nc.dram_tensor("out_t", [R, ELEM], f32, kind="ExternalOutput")

    n_cols_of_window = {}
    for w, _, n, _, _ in chunks:
        n_cols_of_window[w] = n_cols_of_window.get(w, 0) + n // 128

    with tile.TileContext(nc) as tc:
        with (
            tc.tile_pool(name="gt", bufs=3) as gp,
            tc.tile_pool(name="aux", bufs=6) as auxp,
            tc.tile_pool(name="oh", bufs=4) as ohp,
            tc.tile_pool(name="ot", bufs=3) as otp,
            tc.tile_pool(name="psum", bufs=6, space="PSUM") as psp,
            tc.tile_pool(name="const", bufs=1) as cp,
        ):
            iota = cp.tile([128, 128], f32)
            nc.gpsimd.iota(
                iota[:], pattern=[[1, 128]], base=0, channel_multiplier=0,
                allow_small_or_imprecise_dtypes=True,
            )

            off = 0
            acc = None
            col_of_window = 0
            for w, bank_b, n, first, last in chunks:
                S, C = n // 16, n // 128
                so, co = off // 16, off // 128
                off += n
                gi = auxp.tile([128, S], i16, tag="gi")
                rr = auxp.tile([128, C], f32, tag="rr")
                vt = auxp.tile([128, C, BATCH], f32, tag="vt")
                nc.sync.dma_start(gi[:], g_idx[:, so : so + S])
                nc.sync.dma_start(rr[:], relrow[:, co : co + C])
                nc.sync.dma_start(vt[:], vals[:, co : co + C])

                gt = gp.tile([128, C, ELEM], f32, tag="gt")
                src = b_t[0:BANK] if not bank_b else b_t[BANK:N_NODES]
                for c0 in range(0, C, GCAP // 128):
                    c1 = min(c0 + GCAP // 128, C)
                    nsub = (c1 - c0) * 128
                    nc.gpsimd.dma_gather(
                        gt[:, c0:c1, :], src,
                        gi[:, c0 * 8 : c0 * 8 + nsub // 16],
                        nsub, nsub, ELEM,
                    )

                for k in range(BATCH):
                    gslice = gt[:, :, k * FEAT : (k + 1) * FEAT]
                    v_ap = vt[:, :, k]
                    v_b = bass.AP(v_ap.tensor, v_ap.offset, list(v_ap.ap) + [[0, FEAT]])
                    nc.vector.tensor_mul(gslice, gslice, v_b)

                if first:
                    acc = psp.tile([128, ELEM], f32, tag="acc")
                    col_of_window = 0
                for c in range(C):
                    oh = ohp.tile([128, 128], f32, tag="oh")
                    nc.vector.tensor_scalar(
                        oh[:], iota[:], rr[:, c : c + 1], None,
                        mybir.AluOpType.is_equal,
                    )
                    nc.tensor.matmul(
                        acc[:], oh[:], gt[:, c, :],
                        start=(col_of_window == 0),
                        stop=(col_of_window == n_cols_of_window[w] - 1),
                    )
                    col_of_window += 1

                if last:
                    r0 = w * W
                    r1 = min(r0 + W, R)
                    ot = otp.tile([128, ELEM], f32, tag="ot")
                    nc.vector.tensor_copy(ot[:], acc[:])
                    nc.sync.dma_start(out_t[r0:r1], ot[: r1 - r0])

    nc.compile()
    return nc


# ---------------------------------------------------------------- entry point
def _prepare(indices, values):
    row = np.asarray(indices[0], np.int64)
    col = np.asarray(indices[1], np.int64)
    values = np.asarray(values, np.float32)

    per_core_edges = []
    per_core_vals = []
    for c in range(N_CORES):
        m = (row // ROWS_PER_CORE) == c
        per_core_edges.append((row[m] - c * ROWS_PER_CORE, col[m]))
        per_core_vals.append(values[:, m])

    chunks, per_core_tokens = _make_structure(per_core_edges)
    packs = [
        _pack_core_inputs(per_core_tokens[c], per_core_vals[c], chunks)
        for c in range(N_CORES)
    ]
    return chunks, packs


def _get_program(indices, values):
    key = hashlib.sha1(np.ascontiguousarray(indices).tobytes()).hexdigest()
    if key not in _cache:
        from concourse.bass_interp import get_hw_module

        chunks, packs = _prepare(indices, values)
        nc = _build(chunks)
        hw_m = get_hw_module(nc.m)
        _cache[key] = (nc, hw_m, chunks, packs)
    return _cache[key]


def kernel(indices, values, shape_m, shape_n, b):
    import concourse.bass_utils as bass_utils

    indices = np.asarray(indices)
    b = np.asarray(b, np.float32)
    assert int(shape_m) == N_NODES and int(shape_n) == N_NODES
    assert b.shape == (BATCH, N_NODES, FEAT)

    nc, hw_m, chunks, packs = _get_program(indices, values)
    b_t = np.ascontiguousarray(b.transpose(1, 0, 2).reshape(N_NODES, ELEM))
    in_maps = [{"b_t": b_t, **packs[c]} for c in range(N_CORES)]

    old_m = nc.m
    nc.m = hw_m
    try:
        res = bass_utils.run_bass_kernel_spmd(
            nc, in_maps, core_ids=list(range(N_CORES))
        )
    finally:
        nc.m = old_m

    out = np.empty((BATCH, N_NODES, FEAT), np.float32)
    for c in range(N_CORES):
        o = res.results[c]["out_t"]  # [R, ELEM]
        out[:, c * ROWS_PER_CORE : (c + 1) * ROWS_PER_CORE, :] = (
            o.reshape(ROWS_PER_CORE, BATCH, FEAT).transpose(1, 0, 2)
        )
    return out



# revision 2
# speedup vs baseline: 3.2454x; 3.2454x over previous
"""Batched COO SpMM (gnn_message_passing) on TRN2.

out[k, i, :] = sum_{e: row[e]==i} values[k, e] * b[k, col[e], :]
  indices [2, 800000] int32, values [4, 800000] f32, b [4, 50000, 64] f32.

Design: the whole problem runs in ONE device program on ONE NeuronCore.
(Multi-core SPMD dispatch is serialized by the runtime at ~1.5-4.5ms per
device execution, dwarfing the ~0.5ms/core of actual device work — one
execution of an 8x-bigger program is much faster end-to-end.)

Batch-fused element layout: b_t[node, k*64+f] = b[k, node, f] -> 1KB per
node row, so ONE dma_gather descriptor fetches an edge's source features
for all 4 batches. Edges (tokens) are grouped by 128-row output window;
per window, tokens split into col-bank sections (int16 gather index
limit), col-sorted, padded to multiples of 128.

Per chunk: dma_gather from b_t -> gt [128 tokens/col-group, C, 256];
DVE multiplies each token block by its per-batch edge values; per
128-token column DVE builds a one-hot lhsT[t, m] = (relrow[t] == m) and
PE accumulates psum[m,:] += lhsT^T @ gt[:,c,:] over the window's columns.
Window end: PSUM -> SBUF -> DMA to out_t rows. No scatter, no RMW.
Pad tokens: gather node 0, values 0, relrow -1 (one-hot row all zero).
"""
import hashlib

import numpy as np

N_NODES = 50000
NNZ = 800000
BATCH = 4
FEAT = 64
ELEM = BATCH * FEAT
N_CORES = 1  # whole problem on one core (see module docstring)
ROWS_PER_CORE = N_NODES
BANK = 32768
W = 128  # output rows per PSUM window

_cache = {}


# ---------------------------------------------------------------- host prep
def _make_structure(rows, cols):
    """Group edges by 128-row output window; per window split into col banks
    (int16 index limit), col-sort, pad to multiples of 128."""
    NW = -(-ROWS_PER_CORE // W)
    win = rows // W
    order = np.argsort(win, kind="stable")
    bounds = np.searchsorted(win[order], np.arange(NW + 1))
    sections = []
    for w in range(NW):
        in_w = order[bounds[w] : bounds[w + 1]]
        cw = cols[in_w]
        a = in_w[cw < BANK]
        b = in_w[cw >= BANK]
        a = a[np.argsort(cols[a], kind="stable")]
        b = b[np.argsort(cols[b], kind="stable")]
        sections.append((a, b))

    chunks = []
    for w in range(NW):
        nA = max(-(-len(sections[w][0]) // 128) * 128, 128)
        nB = -(-len(sections[w][1]) // 128) * 128
        parts = [(w, 0, nA)] + ([(w, 1, nB)] if nB else [])
        for i, (w_, b_, n_) in enumerate(parts):
            chunks.append((w_, b_, n_, i == 0, i == len(parts) - 1))

    g_parts, r_parts, e_parts = [], [], []
    for w, bank_b, n, _, _ in chunks:
        sel = sections[w][bank_b]
        k = len(sel)
        g = np.zeros(n, np.int16)
        rr = np.full(n, -1.0, np.float32)
        e = np.full(n, -1, np.int64)
        g[:k] = (cols[sel] - (BANK if bank_b else 0)).astype(np.int16)
        rr[:k] = (rows[sel] - w * W).astype(np.float32)
        e[:k] = sel
        g_parts.append(g)
        r_parts.append(rr)
        e_parts.append(e)
    tokens = {
        "g": np.concatenate(g_parts),
        "rr": np.concatenate(r_parts),
        "e": np.concatenate(e_parts),
    }
    return chunks, tokens


def _pack_inputs(tokens, values_be, chunks):
    g_cols, r_cols, v_cols = [], [], []
    off = 0
    for _, _, n, _, _ in chunks:
        g = tokens["g"][off : off + n]
        rr = tokens["rr"][off : off + n]
        e = tokens["e"][off : off + n]
        off += n
        g_cols.append(g.reshape(-1, 16).T)
        r_cols.append(rr.reshape(-1, 128).T)
        v = np.zeros((n, BATCH), np.float32)
        real = e >= 0
        v[real] = values_be[:, e[real]].T
        v_cols.append(v.reshape(-1, 128, BATCH).transpose(1, 0, 2))
    g_idx = np.tile(np.concatenate(g_cols, axis=1), (8, 1)).astype(np.int16)
    relrow = np.concatenate(r_cols, axis=1).astype(np.float32)
    vals = np.ascontiguousarray(np.concatenate(v_cols, axis=1).astype(np.float32))
    return {
        "g_idx": np.ascontiguousarray(g_idx),
        "relrow": np.ascontiguousarray(relrow),
        "vals": vals,
    }


# HW constraint: a single dma_gather instruction with more than ~1300
# descriptors crashes the core (NRT_EXEC_UNIT_UNRECOVERABLE). Split large
# gathers into sub-instructions of at most GCAP tokens.
GCAP = 1024


# ---------------------------------------------------------------- device code
def _build(chunks):
    import concourse.bacc as bacc
    import concourse.bass as bass
    import concourse.mybir as mybir
    import concourse.tile as tile

    f32 = mybir.dt.float32
    i16 = mybir.dt.int16
    T = sum(c[2] for c in chunks)
    S_total, C_total = T // 16, T // 128
    R = ROWS_PER_CORE

    nc = bacc.Bacc(None, target_bir_lowering=False)
    b_t = nc.dram_tensor("b_t", [N_NODES, ELEM], f32, kind="ExternalInput")
    g_idx = nc.dram_tensor("g_idx", [128, S_total], i16, kind="ExternalInput")
    relrow = nc.dram_tensor("relrow", [128, C_total], f32, kind="ExternalInput")
    vals = nc.dram_tensor("vals", [128, C_total, BATCH], f32, kind="ExternalInput")
    out_t = nc.dram_tensor("out_t", [R, ELEM], f32, kind="ExternalOutput")

    n_cols_of_window = {}
    for w, _, n, _, _ in chunks:
        n_cols_of_window[w] = n_cols_of_window.get(w, 0) + n // 128

    with tile.TileContext(nc) as tc:
        with (
            tc.tile_pool(name="gt", bufs=3) as gp,
            tc.tile_pool(name="aux", bufs=6) as auxp,
            tc.tile_pool(name="oh", bufs=4) as ohp,
            tc.tile_pool(name="ot", bufs=3) as otp,
            tc.tile_pool(name="psum", bufs=6, space="PSUM") as psp,
            tc.tile_pool(name="const", bufs=1) as cp,
        ):
            iota = cp.tile([128, 128], f32)
            nc.gpsimd.iota(
                iota[:], pattern=[[1, 128]], base=0, channel_multiplier=0,
                allow_small_or_imprecise_dtypes=True,
            )

            off = 0
            acc = None
            col_of_window = 0
            for w, bank_b, n, first, last in chunks:
                S, C = n // 16, n // 128
                so, co = off // 16, off // 128
                off += n
                gi = auxp.tile([128, S], i16, tag="gi")
                rr = auxp.tile([128, C], f32, tag="rr")
                vt = auxp.tile([128, C, BATCH], f32, tag="vt")
                nc.sync.dma_start(gi[:], g_idx[:, so : so + S])
                nc.sync.dma_start(rr[:], relrow[:, co : co + C])
                nc.sync.dma_start(vt[:], vals[:, co : co + C])

                gt = gp.tile([128, C, ELEM], f32, tag="gt")
                src = b_t[0:BANK] if not bank_b else b_t[BANK:N_NODES]
                for c0 in range(0, C, GCAP // 128):
                    c1 = min(c0 + GCAP // 128, C)
                    nsub = (c1 - c0) * 128
                    nc.gpsimd.dma_gather(
                        gt[:, c0:c1, :], src,
                        gi[:, c0 * 8 : c0 * 8 + nsub // 16],
                        nsub, nsub, ELEM,
                    )

                for k in range(BATCH):
                    gslice = gt[:, :, k * FEAT : (k + 1) * FEAT]
                    v_ap = vt[:, :, k]
                    v_b = bass.AP(v_ap.tensor, v_ap.offset, list(v_ap.ap) + [[0, FEAT]])
                    nc.vector.tensor_mul(gslice, gslice, v_b)

                if first:
                    acc = psp.tile([128, ELEM], f32, tag="acc")
                    col_of_window = 0
                for c in range(C):
                    oh = ohp.tile([128, 128], f32, tag="oh")
                    nc.vector.tensor_scalar(
                        oh[:], iota[:], rr[:, c : c + 1], None,
                        mybir.AluOpType.is_equal,
                    )
                    nc.tensor.matmul(
                        acc[:], oh[:], gt[:, c, :],
                        start=(col_of_window == 0),
                        stop=(col_of_window == n_cols_of_window[w] - 1),
                    )
                    col_of_window += 1

                if last:
                    r0 = w * W
                    r1 = min(r0 + W, R)
                    ot = otp.tile([128, ELEM], f32, tag="ot")
                    nc.vector.tensor_copy(ot[:], acc[:])
                    nc.sync.dma_start(out_t[r0:r1], ot[: r1 - r0])

    nc.compile()
    return nc


# ---------------------------------------------------------------- entry point
def _prepare(indices, values):
    row = np.asarray(indices[0], np.int64)
    col = np.asarray(indices[1], np.int64)
    values = np.asarray(values, np.float32)
    chunks, tokens = _make_structure(row, col)
    packs = [_pack_inputs(tokens, values, chunks)]
    return chunks, packs


def _get_program(indices, values):
    key = (
        hashlib.sha1(np.ascontiguousarray(indices).tobytes()).hexdigest()
        + hashlib.sha1(np.ascontiguousarray(values).tobytes()).hexdigest()
    )
    if key not in _cache:
        from concourse.bass_interp import get_hw_module

        chunks, packs = _prepare(indices, values)
        nc = _build(chunks)
        hw_m = get_hw_module(nc.m)
        _cache[key] = (nc, hw_m, chunks, packs)
    return _cache[key]


def kernel(indices, values, shape_m, shape_n, b):
    import concourse.bass_utils as bass_utils

    indices = np.asarray(indices)
    b = np.asarray(b, np.float32)
    assert int(shape_m) == N_NODES and int(shape_n) == N_NODES
    assert b.shape == (BATCH, N_NODES, FEAT)

    nc, hw_m, chunks, packs = _get_program(indices, values)
    b_t = np.ascontiguousarray(b.transpose(1, 0, 2).reshape(N_NODES, ELEM))
    in_maps = [{"b_t": b_t, **packs[0]}]

    old_m = nc.m
    nc.m = hw_m
    try:
        res = bass_utils.run_bass_kernel_spmd(nc, in_maps, core_ids=[0])
    finally:
        nc.m = old_m

    o = res.results[0]["out_t"]  # [N_NODES, ELEM]
    return np.ascontiguousarray(
        o.reshape(N_NODES, BATCH, FEAT).transpose(1, 0, 2)
    )
